# revision 5
# baseline (speedup 1.0000x reference)
"""Trainium2 Bass kernel for masked attention scoring (sparse_attention).

Computes, per batch b:
    proj = y @ M^T                      # [B, D]
    eij  = tanh(einsum('bsd,bd->bs', x, proj))
    a    = exp(eij) * mask
    a    = a / (sum_s a + EPS)

Sharding: data-parallel over batch B=32 across 8 NeuronCores (4 batches
per core). M is replicated; all reductions stay local per shard.

Per-core device algorithm (memory-bound, x-stream dominated):
  - proj: transpose M via TensorE (64 identity-matmul blocks), then
    matmul yT @ M^T accumulated in PSUM, staged to a DRAM scratch and
    broadcast-DMA'd across 128 partitions per batch.
  - main pass: stream x in [128, 8, 1024] tiles (natural layout, 4 MiB
    DMAs) and compute the d-reduction with ONE fused DVE op per s-chunk:
    tensor_tensor_reduce(mult, add) -> eij column.
  - epilogue: tanh+exp on ScalarE, mask multiply, free-dim reduce on
    VectorE, partition reduce + scalar broadcast via tiny TensorE
    matmuls with ones, normalize, strided DMA out.
"""

import os
import sys

import numpy as np

for _p in ("/opt/trn_rl_repo",):
    if os.path.isdir(_p) and _p not in sys.path:
        sys.path.insert(0, _p)

B, S, D = 32, 2048, 1024
NCORES = 8
BL = B // NCORES        # batches per core
P = 128                 # SBUF partitions
J = S // P              # 16 s-chunks per batch
HALF = J // 2           # s-chunks per x DMA (4 MiB)
DC = D // P             # 8 d-chunks
EPS = 1e-7

_CACHE = {}


def _build():
    import concourse.bacc as bacc
    import concourse.tile as tile
    from concourse import mybir
    from concourse.masks import make_identity

    f32 = mybir.dt.float32
    i32 = mybir.dt.int32

    nc = bacc.Bacc("TRN2", target_bir_lowering=False, debug=False,
                   num_devices=NCORES)

    x_ext = nc.dram_tensor("x", [BL, S, D], f32, kind="ExternalInput").ap()
    y_ext = nc.dram_tensor("y", [BL, D], f32, kind="ExternalInput").ap()
    mask_ext = nc.dram_tensor("mask", [BL, S], i32, kind="ExternalInput").ap()
    m_ext = nc.dram_tensor("M", [D, D], f32, kind="ExternalInput").ap()
    out_ext = nc.dram_tensor("out", [BL, S], f32, kind="ExternalOutput").ap()
    proj_scr = nc.dram_tensor("proj_scr", [BL, D], f32).ap()

    with tile.TileContext(nc) as tc:
        with (
            tc.tile_pool(name="consts", bufs=1) as consts,
            tc.tile_pool(name="mnat", bufs=2) as mnat_pool,
            tc.tile_pool(name="mtsb", bufs=3) as mtsb_pool,
            tc.tile_pool(name="psum_t", bufs=2, space="PSUM") as psum_t_pool,
            tc.tile_pool(name="psum_proj", bufs=1, space="PSUM") as psum_proj_pool,
            tc.tile_pool(name="psum_misc", bufs=2, space="PSUM") as psum_misc_pool,
            tc.tile_pool(name="projbc", bufs=1) as projbc_pool,
            tc.tile_pool(name="xpool", bufs=3) as xpool,
            tc.tile_pool(name="scr", bufs=2) as scr_pool,
            tc.tile_pool(name="eij", bufs=2) as eij_pool,
            tc.tile_pool(name="ep", bufs=2) as ep_pool,
        ):
            identity = consts.tile([P, P], f32)
            make_identity(nc, identity)
            ones_col = consts.tile([P, 1], f32)
            nc.vector.memset(ones_col, 1.0)
            ones_row = consts.tile([1, P], f32)
            nc.vector.memset(ones_row, 1.0)
            eps_t = consts.tile([1, 1], f32)
            nc.vector.memset(eps_t, EPS)

            # ---- proj = y @ M^T, all BL batches at once ----
            yT = consts.tile([P, DC, BL], f32)
            for dc in range(DC):
                nc.gpsimd.dma_start(
                    out=yT[:, dc, :],
                    in_=y_ext[:, dc * P:(dc + 1) * P].rearrange("b p -> p b"))

            proj_ps = psum_proj_pool.tile([BL, D], f32)
            for ec in range(DC):
                mnat = mnat_pool.tile([P, D], f32, tag="mnat")
                nc.scalar.dma_start(out=mnat, in_=m_ext[ec * P:(ec + 1) * P, :])
                for dc in range(DC):
                    pt = psum_t_pool.tile([P, P], f32, tag="pt")
                    nc.tensor.transpose(pt, mnat[:, dc * P:(dc + 1) * P],
                                        identity)
                    mtb = mtsb_pool.tile([P, P], f32, tag="mtb")
                    nc.scalar.copy(mtb, pt)
                    nc.tensor.matmul(
                        proj_ps[:, ec * P:(ec + 1) * P],
                        lhsT=yT[:, dc, :],
                        rhs=mtb,
                        start=(dc == 0),
                        stop=(dc == DC - 1),
                    )
            proj_sb = consts.tile([BL, D], f32)
            nc.scalar.copy(proj_sb, proj_ps)
            nc.sync.dma_start(out=proj_scr, in_=proj_sb)

            projbc = []
            for b in range(BL):
                pb = projbc_pool.tile([P, D], f32, tag=f"projbc{b}")
                nc.gpsimd.dma_start(out=pb,
                                    in_=proj_scr[b].partition_broadcast(P))
                projbc.append(pb)

            # ---- main pass: eij[s] = sum_d x[s, d] * proj[d] ----
            for b in range(BL):
                eij = eij_pool.tile([P, J], f32, tag="eij")
                for half in range(2):
                    xt = xpool.tile([P, HALF, D], f32, tag="xt")
                    nc.sync.dma_start(
                        out=xt,
                        in_=x_ext[b, half * HALF * P:(half + 1) * HALF * P, :]
                        .rearrange("(j p) d -> p j d", p=P),
                    )
                    for j in range(HALF):
                        col = half * HALF + j
                        scr = scr_pool.tile([P, D], f32, tag="scr")
                        nc.vector.scalar_tensor_tensor(
                            out=scr,
                            in0=xt[:, j, :],
                            scalar=1.0,
                            in1=projbc[b],
                            op0=mybir.AluOpType.mult,
                            op1=mybir.AluOpType.mult,
                            accum_out=eij[:, col:col + 1],
                        )

                # ---- epilogue: a = exp(tanh(eij)) * mask; normalize ----
                th = ep_pool.tile([P, J], f32, tag="th")
                nc.scalar.activation(th, eij,
                                     mybir.ActivationFunctionType.Tanh)
                ex = ep_pool.tile([P, J], f32, tag="ex")
                nc.scalar.activation(ex, th, mybir.ActivationFunctionType.Exp)
                mk = ep_pool.tile([P, J], f32, tag="mk")
                nc.gpsimd.dma_start(
                    out=mk, in_=mask_ext[b].rearrange("(j p) -> p j", p=P))
                au = ep_pool.tile([P, J], f32, tag="au")
                nc.vector.tensor_mul(au, ex, mk)
                cs = ep_pool.tile([P, 1], f32, tag="cs")
                nc.vector.reduce_sum(cs, au, axis=mybir.AxisListType.X)
                tot_ps = psum_misc_pool.tile([1, 1], f32, tag="misc")
                nc.tensor.matmul(tot_ps, lhsT=cs, rhs=ones_col,
                                 start=True, stop=True)
                tot_sb = ep_pool.tile([1, 1], f32, tag="tots")
                nc.scalar.activation(tot_sb, tot_ps,
                                     mybir.ActivationFunctionType.Identity,
                                     bias=eps_t, scale=1.0)
                rec = ep_pool.tile([1, 1], f32, tag="rec")
                nc.vector.reciprocal(rec, tot_sb)
                rbc_ps = psum_misc_pool.tile([P, 1], f32, tag="misc")
                nc.tensor.matmul(rbc_ps, lhsT=ones_row, rhs=rec,
                                 start=True, stop=True)
                rbc_sb = ep_pool.tile([P, 1], f32, tag="rbcs")
                nc.scalar.copy(rbc_sb, rbc_ps)
                an = ep_pool.tile([P, J], f32, tag="an")
                nc.scalar.mul(an, au, rbc_sb)
                nc.sync.dma_start(
                    out=out_ext[b].rearrange("(j p) -> p j", p=P), in_=an)

    nc.compile()
    return nc


def _get_nc():
    if "nc" not in _CACHE:
        _CACHE["nc"] = _build()
    return _CACHE["nc"]


def kernel(x, y, mask, M, **_ignored):
    from concourse.bass_utils import run_bass_kernel_spmd

    x = np.ascontiguousarray(np.asarray(x, dtype=np.float32))
    y = np.ascontiguousarray(np.asarray(y, dtype=np.float32))
    mask = np.ascontiguousarray(np.asarray(mask, dtype=np.int32))
    M = np.ascontiguousarray(np.asarray(M, dtype=np.float32))

    nc = _get_nc()
    in_maps = [
        {
            "x": x[i * BL:(i + 1) * BL],
            "y": y[i * BL:(i + 1) * BL],
            "mask": mask[i * BL:(i + 1) * BL],
            "M": M,
        }
        for i in range(NCORES)
    ]
    res = run_bass_kernel_spmd(nc, in_maps, core_ids=list(range(NCORES)))
    out = np.concatenate([res.results[i]["out"] for i in range(NCORES)],
                         axis=0)
    return out.astype(np.float32)


# revision 9
# speedup vs baseline: 1.0997x; 1.0997x over previous
"""Trainium2 Bass kernel for masked attention scoring (sparse_attention).

Computes, per batch b:
    proj = y @ M^T                      # [B, D]
    eij  = tanh(einsum('bsd,bd->bs', x, proj))
    a    = exp(eij) * mask
    a    = a / (sum_s a + EPS)

Sharding: data-parallel over batch B=32 across 8 NeuronCores (4 batches
per core). M is replicated; all reductions stay local per shard.

Per-core device algorithm (memory-bound, x-stream dominated):
  - proj: M chunks cast-DMA'd to bf16, transposed on TensorE (batched
    8 blocks per PSUM bank), proj = yT^T @ M^T accumulated in PSUM
    (bf16 inputs, f32 accum), then broadcast across 128 partitions via
    ones-vector matmuls on TensorE (no DRAM round trip).
  - main pass: stream x in [128, 8, 1024] f32 tiles (natural layout,
    4 MiB DMAs) and compute the d-reduction with ONE fused DVE op per
    s-chunk: scalar_tensor_tensor(mult, mult, accum_out) -> eij column.
  - epilogue (once, after all batches): tanh+exp on ScalarE over
    [128, 64], mask multiply, free-dim reduce on VectorE, partition
    reduce + denominator broadcast via tiny TensorE matmuls with ones,
    normalize, one strided DMA out.
"""

import os
import sys

import numpy as np

for _p in ("/opt/trn_rl_repo",):
    if os.path.isdir(_p) and _p not in sys.path:
        sys.path.insert(0, _p)

B, S, D = 32, 2048, 1024
NCORES = 8
BL = B // NCORES        # batches per core
P = 128                 # SBUF partitions
J = S // P              # 16 s-chunks per batch
HALF = J // 2           # s-chunks per x DMA (4 MiB)
DC = D // P             # 8 d-chunks
EPS = 1e-7

_CACHE = {}


def _build():
    import concourse.bacc as bacc
    import concourse.tile as tile
    from concourse import mybir
    from concourse.masks import make_identity

    f32 = mybir.dt.float32
    bf16 = mybir.dt.float16
    i32 = mybir.dt.int32

    nc = bacc.Bacc("TRN2", target_bir_lowering=False, debug=False,
                   num_devices=NCORES)

    x_ext = nc.dram_tensor("x", [BL, S, D], f32, kind="ExternalInput").ap()
    y_ext = nc.dram_tensor("y", [BL, D], f32, kind="ExternalInput").ap()
    mask_ext = nc.dram_tensor("mask", [BL, S], i32, kind="ExternalInput").ap()
    m_ext = nc.dram_tensor("M", [D, D], f32, kind="ExternalInput").ap()
    out_ext = nc.dram_tensor("out", [BL, S], f32, kind="ExternalOutput").ap()

    with tile.TileContext(nc) as tc:
        with (
            tc.tile_pool(name="consts", bufs=1) as consts,
            tc.tile_pool(name="mnat", bufs=8) as mnat_pool,
            tc.tile_pool(name="psum_t", bufs=2, space="PSUM") as psum_t_pool,
            tc.tile_pool(name="psum_proj", bufs=1, space="PSUM") as psum_proj_pool,
            tc.tile_pool(name="psum_pb", bufs=2, space="PSUM") as psum_pb_pool,
            tc.tile_pool(name="psum_misc", bufs=2, space="PSUM") as psum_misc_pool,
            tc.tile_pool(name="xpool", bufs=4) as xpool,
            tc.tile_pool(name="scr", bufs=2) as scr_pool,
        ):
            identity_bf = consts.tile([P, P], bf16)
            make_identity(nc, identity_bf)
            ones_col = consts.tile([P, 1], f32)
            nc.vector.memset(ones_col, 1.0)
            ones_row = consts.tile([1, P], f32)
            nc.vector.memset(ones_row, 1.0)
            eps_t = consts.tile([1, 1], f32)
            nc.vector.memset(eps_t, EPS)

            # masks for all batches, int32 -> f32 cast during DMA,
            # laid out as [p, b, j] to match eij
            mask_all = consts.tile([P, BL, J], f32)
            for b in range(BL):
                nc.gpsimd.dma_start(
                    out=mask_all[:, b, :],
                    in_=mask_ext[b].rearrange("(j p) -> p j", p=P))

            # yT in bf16: yT[p, dc, b] = y[b, dc*128+p]
            yT = consts.tile([P, DC, BL], bf16)
            for dc in range(DC):
                nc.gpsimd.dma_start(
                    out=yT[:, dc, :],
                    in_=y_ext[:, dc * P:(dc + 1) * P].rearrange("b p -> p b"))

            # ---- M^T in bf16 via TensorE transposes ----
            # mtsb[p_d, ec, dc, e'] = M[ec*128+e', dc*128+p_d]
            mtsb = consts.tile([P, DC, DC, P], bf16)
            for ec in range(DC):
                mnat = mnat_pool.tile([P, D], bf16, tag="mnat")
                nc.gpsimd.dma_start(out=mnat,
                                    in_=m_ext[ec * P:(ec + 1) * P, :])
                pt = psum_t_pool.tile([P, DC, P], bf16, tag="pt")
                for dc in range(DC):
                    nc.tensor.transpose(pt[:, dc, :],
                                        mnat[:, dc * P:(dc + 1) * P],
                                        identity_bf)
                nc.scalar.copy(mtsb[:, ec, :, :], pt)

            # ---- proj[b, e] = sum_d y[b, d] * M[e, d]  (PSUM f32) ----
            proj_ps = psum_proj_pool.tile([BL, D], f32)
            for dc in range(DC):
                for eh in range(2):
                    # rhs free dims: (ec within half, e') -> e contiguous
                    nc.tensor.matmul(
                        proj_ps[:, eh * 512:(eh + 1) * 512],
                        lhsT=yT[:, dc, :],
                        rhs=mtsb[:, eh * 4:(eh + 1) * 4, dc, :],
                        start=(dc == 0),
                        stop=(dc == DC - 1),
                    )
            proj_sb = consts.tile([BL, D], f32)
            nc.scalar.copy(proj_sb, proj_ps)

            # ---- broadcast proj rows across partitions via TensorE ----
            # sel[b] is [BL, P] with row b all-ones: sel[b].T @ proj_sb
            # replicates proj row b onto all 128 output partitions.
            projbc = []
            for b in range(BL):
                sel = consts.tile([BL, P], f32, name=f"sel{b}")
                nc.gpsimd.memset(sel, 0.0)
                # iota = partition - b; != 0 keeps 0.0, == 0 fills 1.0
                nc.gpsimd.affine_select(
                    out=sel, in_=sel,
                    compare_op=mybir.AluOpType.not_equal,
                    fill=1.0, base=-b,
                    pattern=[[0, P]], channel_multiplier=1)
                pb = consts.tile([P, D], f32, name=f"projbc{b}")
                for eh in range(2):
                    pb_ps = psum_pb_pool.tile([P, 512], f32, tag="pbps")
                    nc.tensor.matmul(
                        pb_ps,
                        lhsT=sel,
                        rhs=proj_sb[:, eh * 512:(eh + 1) * 512],
                        start=True, stop=True)
                    nc.scalar.copy(pb[:, eh * 512:(eh + 1) * 512], pb_ps)
                projbc.append(pb)

            # ---- main pass: eij[p, b, col] = x[b, s, :] . proj[b, :] ----
            eij = consts.tile([P, BL, J], f32)
            for b in range(BL):
                for half in range(2):
                    xt = xpool.tile([P, HALF, D], f32, tag="xt")
                    nc.sync.dma_start(
                        out=xt,
                        in_=x_ext[b, half * HALF * P:(half + 1) * HALF * P, :]
                        .rearrange("(j p) d -> p j d", p=P),
                    )
                    for j in range(HALF):
                        col = half * HALF + j
                        scr = scr_pool.tile([P, D], f32, tag="scr")
                        nc.vector.scalar_tensor_tensor(
                            out=scr,
                            in0=xt[:, j, :],
                            scalar=1.0,
                            in1=projbc[b],
                            op0=mybir.AluOpType.mult,
                            op1=mybir.AluOpType.mult,
                            accum_out=eij[:, b, col:col + 1],
                        )

            # ---- fused epilogue over all batches ----
            th = consts.tile([P, BL, J], f32)
            nc.scalar.activation(th, eij, mybir.ActivationFunctionType.Tanh)
            ex = consts.tile([P, BL, J], f32)
            nc.scalar.activation(ex, th, mybir.ActivationFunctionType.Exp)
            au = consts.tile([P, BL, J], f32)
            nc.vector.tensor_mul(au, ex, mask_all)
            cs = consts.tile([P, BL], f32)
            nc.vector.reduce_sum(cs, au, axis=mybir.AxisListType.X)
            tot_ps = psum_misc_pool.tile([1, BL], f32, tag="misc")
            nc.tensor.matmul(tot_ps, lhsT=ones_col, rhs=cs,
                             start=True, stop=True)
            tot_sb = consts.tile([1, BL], f32)
            nc.scalar.activation(tot_sb, tot_ps,
                                 mybir.ActivationFunctionType.Identity,
                                 bias=eps_t, scale=1.0)
            rec = consts.tile([1, BL], f32)
            nc.vector.reciprocal(rec, tot_sb)
            rbc_ps = psum_misc_pool.tile([P, BL], f32, tag="misc")
            nc.tensor.matmul(rbc_ps, lhsT=ones_row, rhs=rec,
                             start=True, stop=True)
            rbc_sb = consts.tile([P, BL], f32)
            nc.scalar.copy(rbc_sb, rbc_ps)
            an = consts.tile([P, BL, J], f32)
            for b in range(BL):
                nc.scalar.mul(an[:, b, :], au[:, b, :], rbc_sb[:, b:b + 1])
            nc.sync.dma_start(
                out=out_ext.rearrange("b (j p) -> p b j", p=P), in_=an)

    nc.compile()
    return nc


def _get_nc():
    if "nc" not in _CACHE:
        _CACHE["nc"] = _build()
    return _CACHE["nc"]


def kernel(x, y, mask, M, **_ignored):
    from concourse.bass_utils import run_bass_kernel_spmd

    x = np.ascontiguousarray(np.asarray(x, dtype=np.float32))
    y = np.ascontiguousarray(np.asarray(y, dtype=np.float32))
    mask = np.ascontiguousarray(np.asarray(mask, dtype=np.int32))
    M = np.ascontiguousarray(np.asarray(M, dtype=np.float32))

    nc = _get_nc()
    in_maps = [
        {
            "x": x[i * BL:(i + 1) * BL],
            "y": y[i * BL:(i + 1) * BL],
            "mask": mask[i * BL:(i + 1) * BL],
            "M": M,
        }
        for i in range(NCORES)
    ]
    res = run_bass_kernel_spmd(nc, in_maps, core_ids=list(range(NCORES)))
    out = np.concatenate([res.results[i]["out"] for i in range(NCORES)],
                         axis=0)
    return out.astype(np.float32)


# revision 11
# speedup vs baseline: 1.5340x; 1.3949x over previous
"""Trainium2 Bass kernel for masked attention scoring (sparse_attention).

Computes, per batch b:
    proj = y @ M^T                      # [B, D]
    eij  = tanh(einsum('bsd,bd->bs', x, proj))
    a    = exp(eij) * mask
    a    = a / (sum_s a + EPS)

Sharding: data-parallel over batch B=32 across 8 NeuronCores (4 batches
per core). M is replicated; all reductions stay local per shard.

Per-core device algorithm (memory-bound, x-stream dominated):
  - All HBM bulk traffic rides the sync HWDGE ring; the 8 M-chunk loads
    are queued ahead of the x stream so proj is ready ~20us in.
  - proj: M chunks cast to f16 on VectorE, transposed on TensorE
    (8 blocks batched per PSUM bank), proj = yT^T @ M^T accumulated in
    PSUM f32, broadcast across partitions via selector matmuls.
  - main pass: stream x in [128, 8, 1024] f32 tiles (natural layout,
    4 MiB DMAs) and compute the d-reduction with ONE fused DVE op per
    s-chunk: scalar_tensor_tensor(mult, mult, accum_out) -> eij column.
  - epilogue (once): tanh+exp on ScalarE over [128, 64], mask multiply,
    free-dim reduce, partition reduce + denominator broadcast via tiny
    TensorE matmuls, normalize, PE-transpose, one contiguous DMA out.
  - no strided elementwise DMAs anywhere: mask in and a out go through
    PE transposes so every DMA descriptor is a contiguous >=512B run.
"""

import os
import sys

import numpy as np

for _p in ("/opt/trn_rl_repo",):
    if os.path.isdir(_p) and _p not in sys.path:
        sys.path.insert(0, _p)

B, S, D = 32, 2048, 1024
NCORES = 8
BL = B // NCORES        # batches per core
P = 128                 # SBUF partitions
J = S // P              # 16 s-chunks per batch
HALF = J // 2           # s-chunks per x DMA (4 MiB)
DC = D // P             # 8 d-chunks
EPS = 1e-7

_CACHE = {}


def _build():
    import concourse.bacc as bacc
    import concourse.tile as tile
    from concourse import mybir
    from concourse.masks import make_identity
    from concourse.tile import add_dep_helper

    f32 = mybir.dt.float32
    f16 = mybir.dt.float16
    i32 = mybir.dt.int32

    nc = bacc.Bacc("TRN2", target_bir_lowering=False, debug=False,
                   num_devices=NCORES)

    x_ext = nc.dram_tensor("x", [BL, S, D], f32, kind="ExternalInput").ap()
    y_ext = nc.dram_tensor("y", [BL, D], f32, kind="ExternalInput").ap()
    mask_ext = nc.dram_tensor("mask", [BL, S], i32, kind="ExternalInput").ap()
    m_ext = nc.dram_tensor("M", [D, D], f32, kind="ExternalInput").ap()
    out_ext = nc.dram_tensor("out", [BL, S], f32, kind="ExternalOutput").ap()

    with tile.TileContext(nc) as tc:
        with (
            tc.tile_pool(name="consts", bufs=1) as consts,
            tc.tile_pool(name="mnat", bufs=3) as mnat_pool,
            tc.tile_pool(name="mnat16", bufs=3) as mnat16_pool,
            tc.tile_pool(name="psum_t", bufs=2, space="PSUM") as psum_t_pool,
            tc.tile_pool(name="psum_proj", bufs=1, space="PSUM") as psum_proj_pool,
            tc.tile_pool(name="psum_pb", bufs=1, space="PSUM") as psum_pb_pool,
            tc.tile_pool(name="psum_small", bufs=1, space="PSUM") as psum_small_pool,
            tc.tile_pool(name="xpool", bufs=3) as xpool,
            tc.tile_pool(name="scr", bufs=2) as scr_pool,
        ):
            identity16 = consts.tile([P, P], f16)
            make_identity(nc, identity16)
            identity32 = consts.tile([P, P], f32)
            make_identity(nc, identity32)
            ones_col = consts.tile([P, 1], f32)
            nc.vector.memset(ones_col, 1.0)
            ones_row = consts.tile([1, P], f32)
            nc.vector.memset(ones_row, 1.0)
            eps_t = consts.tile([1, 1], f32)
            nc.vector.memset(eps_t, EPS)

            # ---- M^T in f16: load (sync ring, ahead of x), cast,
            #      transpose on TensorE. mtsb[p_d, ec, dc, e'] ----
            mtsb = consts.tile([P, DC, DC, P], f16)
            m_dmas = []
            for ec in range(DC):
                mnat = mnat_pool.tile([P, D], f32, tag="mnat")
                m_dmas.append(
                    nc.sync.dma_start(out=mnat,
                                      in_=m_ext[ec * P:(ec + 1) * P, :]))
                mnat16 = mnat16_pool.tile([P, D], f16, tag="mnat16")
                nc.vector.tensor_copy(mnat16, mnat)
                pt = psum_t_pool.tile([P, DC, P], f16, tag="pt")
                for dc in range(DC):
                    nc.tensor.transpose(pt[:, dc, :],
                                        mnat16[:, dc * P:(dc + 1) * P],
                                        identity16)
                nc.scalar.copy(mtsb[:, ec, :, :], pt)

            # ---- y: one contiguous load, f16 via transpose+cast ----
            ynat = consts.tile([BL, D], f32)
            nc.sync.dma_start(out=ynat, in_=y_ext)
            yt_ps = psum_small_pool.tile([P, DC, BL], f32, tag="small")
            for dc in range(DC):
                nc.tensor.transpose(yt_ps[:, dc, :],
                                    ynat[:, dc * P:(dc + 1) * P],
                                    identity32[:BL, :BL])
            yT = consts.tile([P, DC, BL], f16)
            nc.scalar.copy(yT, yt_ps)

            # ---- proj[b, e] = sum_d y[b, d] * M[e, d]  (PSUM f32) ----
            proj_ps = psum_proj_pool.tile([BL, D], f32)
            for dc in range(DC):
                for eh in range(2):
                    nc.tensor.matmul(
                        proj_ps[:, eh * 512:(eh + 1) * 512],
                        lhsT=yT[:, dc, :],
                        rhs=mtsb[:, eh * 4:(eh + 1) * 4, dc, :],
                        start=(dc == 0),
                        stop=(dc == DC - 1),
                    )
            proj_sb = consts.tile([BL, D], f32)
            nc.scalar.copy(proj_sb, proj_ps)

            # ---- broadcast proj rows across partitions via TensorE ----
            projbc = []
            for b in range(BL):
                sel = consts.tile([BL, P], f32, name=f"sel{b}")
                nc.gpsimd.memset(sel, 0.0)
                nc.gpsimd.affine_select(
                    out=sel, in_=sel,
                    compare_op=mybir.AluOpType.not_equal,
                    fill=1.0, base=-b,
                    pattern=[[0, P]], channel_multiplier=1)
                pb = consts.tile([P, D], f32, name=f"projbc{b}")
                for eh in range(2):
                    pb_ps = psum_pb_pool.tile([P, 512], f32, tag="pbps")
                    nc.tensor.matmul(
                        pb_ps,
                        lhsT=sel,
                        rhs=proj_sb[:, eh * 512:(eh + 1) * 512],
                        start=True, stop=True)
                    nc.scalar.copy(pb[:, eh * 512:(eh + 1) * 512], pb_ps)
                projbc.append(pb)

            # ---- masks: one contiguous cast-DMA + PE transposes ----
            mk_nat = consts.tile([J, BL, P], f32)
            nc.gpsimd.dma_start(
                out=mk_nat,
                in_=mask_ext.rearrange("b (j p) -> j b p", p=P))
            mask_all = consts.tile([P, BL, J], f32)
            for b in range(BL):
                mk_ps = psum_small_pool.tile([P, J], f32, tag="small")
                nc.tensor.transpose(mk_ps, mk_nat[:, b, :], identity32[:J, :J])
                nc.scalar.copy(mask_all[:, b, :], mk_ps)

            # ---- main pass: eij[p, b, col] = x[b, s, :] . proj[b, :] ----
            eij = consts.tile([P, BL, J], f32)
            first_x_dma = None
            for b in range(BL):
                for half in range(2):
                    xt = xpool.tile([P, HALF, D], f32, tag="xt")
                    xd = nc.sync.dma_start(
                        out=xt,
                        in_=x_ext[b, half * HALF * P:(half + 1) * HALF * P, :]
                        .rearrange("(j p) d -> p j d", p=P),
                    )
                    if first_x_dma is None:
                        first_x_dma = xd
                    for j in range(HALF):
                        col = half * HALF + j
                        scr = scr_pool.tile([P, D], f32, tag="scr")
                        nc.vector.scalar_tensor_tensor(
                            out=scr,
                            in0=xt[:, j, :],
                            scalar=1.0,
                            in1=projbc[b],
                            op0=mybir.AluOpType.mult,
                            op1=mybir.AluOpType.mult,
                            accum_out=eij[:, b, col:col + 1],
                        )

            # keep the M loads ahead of the x flood on the sync ring
            for md in m_dmas:
                add_dep_helper(first_x_dma.ins, md.ins, sync=False,
                               reason="x stream after M loads")

            # ---- fused epilogue over all batches ----
            th = consts.tile([P, BL, J], f32)
            nc.scalar.activation(th, eij, mybir.ActivationFunctionType.Tanh)
            ex = consts.tile([P, BL, J], f32)
            nc.scalar.activation(ex, th, mybir.ActivationFunctionType.Exp)
            au = consts.tile([P, BL, J], f32)
            nc.vector.tensor_mul(au, ex, mask_all)
            cs = consts.tile([P, BL], f32)
            nc.vector.reduce_sum(cs, au, axis=mybir.AxisListType.X)
            tot_ps = psum_small_pool.tile([1, BL], f32, tag="small")
            nc.tensor.matmul(tot_ps, lhsT=ones_col, rhs=cs,
                             start=True, stop=True)
            tot_sb = consts.tile([1, BL], f32)
            nc.scalar.activation(tot_sb, tot_ps,
                                 mybir.ActivationFunctionType.Identity,
                                 bias=eps_t, scale=1.0)
            rec = consts.tile([1, BL], f32)
            nc.vector.reciprocal(rec, tot_sb)
            rbc_ps = psum_small_pool.tile([P, BL], f32, tag="small")
            nc.tensor.matmul(rbc_ps, lhsT=ones_row, rhs=rec,
                             start=True, stop=True)
            rbc_sb = consts.tile([P, BL], f32)
            nc.scalar.copy(rbc_sb, rbc_ps)
            an = consts.tile([P, BL, J], f32)
            for b in range(BL):
                nc.scalar.mul(an[:, b, :], au[:, b, :], rbc_sb[:, b:b + 1])
            # transpose [p, (b j)] -> [(b j), p] so the out DMA is
            # 64 contiguous 512B runs instead of 8192 4B elements
            at_ps = psum_small_pool.tile([BL * J, P], f32, tag="small")
            nc.tensor.transpose(at_ps, an.rearrange("p b j -> p (b j)"),
                                identity32)
            an_t = consts.tile([BL * J, P], f32)
            nc.scalar.copy(an_t, at_ps)
            nc.sync.dma_start(
                out=out_ext.rearrange("b (j p) -> (b j) p", p=P), in_=an_t)

    nc.compile()
    return nc


def _get_nc():
    if "nc" not in _CACHE:
        _CACHE["nc"] = _build()
    return _CACHE["nc"]


def kernel(x, y, mask, M, **_ignored):
    from concourse.bass_utils import run_bass_kernel_spmd

    x = np.ascontiguousarray(np.asarray(x, dtype=np.float32))
    y = np.ascontiguousarray(np.asarray(y, dtype=np.float32))
    mask = np.ascontiguousarray(np.asarray(mask, dtype=np.int32))
    M = np.ascontiguousarray(np.asarray(M, dtype=np.float32))

    nc = _get_nc()
    in_maps = [
        {
            "x": x[i * BL:(i + 1) * BL],
            "y": y[i * BL:(i + 1) * BL],
            "mask": mask[i * BL:(i + 1) * BL],
            "M": M,
        }
        for i in range(NCORES)
    ]
    res = run_bass_kernel_spmd(nc, in_maps, core_ids=list(range(NCORES)))
    out = np.concatenate([res.results[i]["out"] for i in range(NCORES)],
                         axis=0)
    return out.astype(np.float32)


# revision 15
# speedup vs baseline: 1.5663x; 1.0210x over previous
"""Trainium2 Bass kernel for masked attention scoring (sparse_attention).

Computes, per batch b:
    proj = y @ M^T                      # [B, D]
    eij  = tanh(einsum('bsd,bd->bs', x, proj))
    a    = exp(eij) * mask
    a    = a / (sum_s a + EPS)

Sharding: data-parallel over batch B=32 across 8 NeuronCores (4 batches
per core). M is replicated; all reductions stay local per shard.

Per-core device algorithm (memory-bound, x-stream dominated):
  - All HBM bulk traffic rides the sync HWDGE ring; the 8 M-chunk loads
    are queued ahead of the x stream so proj is ready ~20us in.
  - proj: M chunks cast to f16 on VectorE, transposed on TensorE
    (8 blocks batched per PSUM bank), proj = yT^T @ M^T accumulated in
    PSUM f32, broadcast across partitions via selector matmuls.
  - main pass: stream x in [128, 8, 1024] f32 tiles (natural layout,
    4 MiB DMAs) and compute the d-reduction with ONE fused DVE op per
    s-chunk: scalar_tensor_tensor(mult, mult, accum_out) -> eij column.
  - epilogue (once): tanh+exp on ScalarE over [128, 64], mask multiply,
    free-dim reduce, partition reduce + denominator broadcast via tiny
    TensorE matmuls, normalize, PE-transpose, one contiguous DMA out.
  - no strided elementwise DMAs anywhere: mask in and a out go through
    PE transposes so every DMA descriptor is a contiguous >=512B run.
"""

import os
import sys

import numpy as np

for _p in ("/opt/trn_rl_repo",):
    if os.path.isdir(_p) and _p not in sys.path:
        sys.path.insert(0, _p)

B, S, D = 32, 2048, 1024
NCORES = 8
BL = B // NCORES        # batches per core
P = 128                 # SBUF partitions
J = S // P              # 16 s-chunks per batch
HALF = J // 2           # s-chunks per x DMA (4 MiB)
DC = D // P             # 8 d-chunks
EPS = 1e-7

_CACHE = {}


def _build():
    import concourse.bacc as bacc
    import concourse.tile as tile
    from concourse import mybir
    from concourse.masks import make_identity
    from concourse.tile import add_dep_helper

    f32 = mybir.dt.float32
    f16 = mybir.dt.float16
    i32 = mybir.dt.int32

    nc = bacc.Bacc("TRN2", target_bir_lowering=False, debug=False,
                   num_devices=NCORES)

    x_ext = nc.dram_tensor("x", [BL, S, D], f32, kind="ExternalInput").ap()
    y_ext = nc.dram_tensor("y", [BL, D], f32, kind="ExternalInput").ap()
    mask_ext = nc.dram_tensor("mask", [BL, S], i32, kind="ExternalInput").ap()
    m_ext = nc.dram_tensor("M", [D, D], f32, kind="ExternalInput").ap()
    out_ext = nc.dram_tensor("out", [BL, S], f32, kind="ExternalOutput").ap()

    with tile.TileContext(nc) as tc:
        with (
            tc.tile_pool(name="consts", bufs=1) as consts,
            tc.tile_pool(name="mnat", bufs=2) as mnat_pool,
            tc.tile_pool(name="mnat16", bufs=2) as mnat16_pool,
            tc.tile_pool(name="psum_t", bufs=2, space="PSUM") as psum_t_pool,
            tc.tile_pool(name="psum_proj", bufs=1, space="PSUM") as psum_proj_pool,
            tc.tile_pool(name="psum_pb", bufs=1, space="PSUM") as psum_pb_pool,
            tc.tile_pool(name="psum_small", bufs=1, space="PSUM") as psum_small_pool,
            tc.tile_pool(name="xpool", bufs=3) as xpool,
            tc.tile_pool(name="scr", bufs=2) as scr_pool,
        ):
            identity16 = consts.tile([P, P], f16)
            make_identity(nc, identity16)
            identity32 = consts.tile([P, P], f32)
            make_identity(nc, identity32)
            ones_col = consts.tile([P, 1], f32)
            nc.vector.memset(ones_col, 1.0)
            ones_row = consts.tile([1, P], f32)
            nc.vector.memset(ones_row, 1.0)
            eps_t = consts.tile([1, 1], f32)
            nc.vector.memset(eps_t, EPS)

            # ---- M^T in f16: load (sync ring, ahead of x), cast,
            #      transpose on TensorE. mtsb[p_d, ec, dc, e'] ----
            # two 2MB loads (amortize per-DMA completion latency),
            # one-op DVE casts, then 8 f16 transposes per e-chunk.
            mtsb = consts.tile([P, DC, DC, P], f16)
            m_dmas = []
            for mh in range(2):
                ec0 = mh * (DC // 2)
                mnat = mnat_pool.tile([P, DC // 2, D], f32, tag="mnat")
                m_dmas.append(nc.sync.dma_start(
                    out=mnat,
                    in_=m_ext[ec0 * P:(ec0 + DC // 2) * P, :]
                    .rearrange("(ec p) d -> p ec d", p=P)))
                mnat16 = mnat16_pool.tile([P, DC // 2, D], f16, tag="mnat16")
                nc.vector.tensor_copy(mnat16, mnat)
                for eci in range(DC // 2):
                    ec = ec0 + eci
                    pt = psum_t_pool.tile([P, DC, P], f16, tag="pt")
                    for dc in range(DC):
                        nc.tensor.transpose(pt[:, dc, :],
                                            mnat16[:, eci,
                                                   dc * P:(dc + 1) * P],
                                            identity16)
                    nc.scalar.copy(mtsb[:, ec, :, :], pt)

            # ---- y: one contiguous load, f16 via transpose+cast ----
            ynat = consts.tile([BL, D], f32)
            nc.sync.dma_start(out=ynat, in_=y_ext)
            yt_ps = psum_small_pool.tile([P, DC, BL], f32, tag="small")
            for dc in range(DC):
                nc.tensor.transpose(yt_ps[:, dc, :],
                                    ynat[:, dc * P:(dc + 1) * P],
                                    identity32[:BL, :BL])
            yT = consts.tile([P, DC, BL], f16)
            nc.scalar.copy(yT, yt_ps)

            # ---- proj[b, e] = sum_d y[b, d] * M[e, d]  (PSUM f32) ----
            proj_ps = psum_proj_pool.tile([BL, D], f32)
            for dc in range(DC):
                for eh in range(2):
                    nc.tensor.matmul(
                        proj_ps[:, eh * 512:(eh + 1) * 512],
                        lhsT=yT[:, dc, :],
                        rhs=mtsb[:, eh * 4:(eh + 1) * 4, dc, :],
                        start=(dc == 0),
                        stop=(dc == DC - 1),
                    )
            proj_sb = consts.tile([BL, D], f16)
            nc.scalar.copy(proj_sb, proj_ps)

            # ---- broadcast proj rows across partitions via TensorE ----
            projbc = []
            for b in range(BL):
                sel = consts.tile([BL, P], f16, name=f"sel{b}")
                nc.gpsimd.memset(sel, 0.0)
                nc.gpsimd.affine_select(
                    out=sel, in_=sel,
                    compare_op=mybir.AluOpType.not_equal,
                    fill=1.0, base=-b,
                    pattern=[[0, P]], channel_multiplier=1)
                pb = consts.tile([P, D], f32, name=f"projbc{b}")
                for eh in range(2):
                    pb_ps = psum_pb_pool.tile([P, 512], f32, tag="pbps")
                    nc.tensor.matmul(
                        pb_ps,
                        lhsT=sel,
                        rhs=proj_sb[:, eh * 512:(eh + 1) * 512],
                        start=True, stop=True)
                    nc.scalar.copy(pb[:, eh * 512:(eh + 1) * 512], pb_ps)
                projbc.append(pb)

            # ---- masks: one contiguous cast-DMA + PE transposes ----
            mk_nat = consts.tile([J, BL, P], f32)
            nc.gpsimd.dma_start(
                out=mk_nat,
                in_=mask_ext.rearrange("b (j p) -> j b p", p=P))
            mask_all = consts.tile([P, BL, J], f32)
            for b in range(BL):
                mk_ps = psum_small_pool.tile([P, J], f32, tag="small")
                nc.tensor.transpose(mk_ps, mk_nat[:, b, :], identity32[:J, :J])
                nc.scalar.copy(mask_all[:, b, :], mk_ps)

            # ---- main pass: eij[p, b, col] = x[b, s, :] . proj[b, :] ----
            eij = consts.tile([P, BL, J], f32)
            first_x_dma = None
            for b in range(BL):
                for half in range(2):
                    xt = xpool.tile([P, HALF, D], f32, tag="xt")
                    xd = nc.sync.dma_start(
                        out=xt,
                        in_=x_ext[b, half * HALF * P:(half + 1) * HALF * P, :]
                        .rearrange("(j p) d -> p j d", p=P),
                    )
                    if first_x_dma is None:
                        first_x_dma = xd
                    for j in range(HALF):
                        col = half * HALF + j
                        scr = scr_pool.tile([P, D], f32, tag="scr")
                        nc.vector.scalar_tensor_tensor(
                            out=scr,
                            in0=xt[:, j, :],
                            scalar=1.0,
                            in1=projbc[b],
                            op0=mybir.AluOpType.mult,
                            op1=mybir.AluOpType.mult,
                            accum_out=eij[:, b, col:col + 1],
                        )

            # keep the M loads ahead of the x flood on the sync ring
            for md in m_dmas:
                add_dep_helper(first_x_dma.ins, md.ins, sync=False,
                               reason="x stream after M loads")

            # ---- fused epilogue over all batches ----
            th = consts.tile([P, BL, J], f32)
            nc.scalar.activation(th, eij, mybir.ActivationFunctionType.Tanh)
            ex = consts.tile([P, BL, J], f32)
            nc.scalar.activation(ex, th, mybir.ActivationFunctionType.Exp)
            au = consts.tile([P, BL, J], f32)
            nc.vector.tensor_mul(au, ex, mask_all)
            cs = consts.tile([P, BL], f32)
            nc.vector.reduce_sum(cs, au, axis=mybir.AxisListType.X)
            tot_ps = psum_small_pool.tile([1, BL], f32, tag="small")
            nc.tensor.matmul(tot_ps, lhsT=ones_col, rhs=cs,
                             start=True, stop=True)
            tot_sb = consts.tile([1, BL], f32)
            nc.scalar.activation(tot_sb, tot_ps,
                                 mybir.ActivationFunctionType.Identity,
                                 bias=eps_t, scale=1.0)
            rec = consts.tile([1, BL], f32)
            nc.vector.reciprocal(rec, tot_sb)
            rbc_ps = psum_small_pool.tile([P, BL], f32, tag="small")
            nc.tensor.matmul(rbc_ps, lhsT=ones_row, rhs=rec,
                             start=True, stop=True)
            rbc_sb = consts.tile([P, BL], f32)
            nc.scalar.copy(rbc_sb, rbc_ps)
            an = consts.tile([P, BL, J], f32)
            for b in range(BL):
                nc.scalar.mul(an[:, b, :], au[:, b, :], rbc_sb[:, b:b + 1])
            # transpose [p, (b j)] -> [(b j), p] so the out DMA is
            # 64 contiguous 512B runs instead of 8192 4B elements
            at_ps = psum_small_pool.tile([BL * J, P], f32, tag="small")
            nc.tensor.transpose(at_ps, an.rearrange("p b j -> p (b j)"),
                                identity32)
            an_t = consts.tile([BL * J, P], f32)
            nc.scalar.copy(an_t, at_ps)
            nc.sync.dma_start(
                out=out_ext.rearrange("b (j p) -> (b j) p", p=P), in_=an_t)

    nc.compile()
    return nc


def _get_nc():
    if "nc" not in _CACHE:
        _CACHE["nc"] = _build()
    return _CACHE["nc"]


def kernel(x, y, mask, M, **_ignored):
    from concourse.bass_utils import run_bass_kernel_spmd

    x = np.ascontiguousarray(np.asarray(x, dtype=np.float32))
    y = np.ascontiguousarray(np.asarray(y, dtype=np.float32))
    mask = np.ascontiguousarray(np.asarray(mask, dtype=np.int32))
    M = np.ascontiguousarray(np.asarray(M, dtype=np.float32))

    nc = _get_nc()
    in_maps = [
        {
            "x": x[i * BL:(i + 1) * BL],
            "y": y[i * BL:(i + 1) * BL],
            "mask": mask[i * BL:(i + 1) * BL],
            "M": M,
        }
        for i in range(NCORES)
    ]
    res = run_bass_kernel_spmd(nc, in_maps, core_ids=list(range(NCORES)))
    out = np.concatenate([res.results[i]["out"] for i in range(NCORES)],
                         axis=0)
    return out.astype(np.float32)
